# revision 1
# baseline (speedup 1.0000x reference)
"""Trainium2 Bass kernel for BiochemicalDynamics.

Reference computation (f32):
    Ax    = A @ x                                   # [N, DIM]
    s     = R * rowsum(x * Ax)                      # [N, 1]
    out   = F - B*x - s                             # [N, DIM]

Key identity used on-device: the output only needs the per-row scalar
    s_i = R * sum_j A[i,j] * <x_i, x_j> = R * rowsum_j (A ⊙ G)[i,j]
with G = x @ x.T. G tiles are produced on the TensorEngine from xT
(stationary xT[:, rows_i], moving xT[:, cols_j]) — so A is consumed in
its natural row-major layout and never needs a transpose. A single
fused VectorEngine op (tensor_tensor_reduce) multiplies the A chunk by
the G chunk from PSUM and row-reduces it, chaining the per-partition
accumulator across chunks.

Sharding: row-shard A (and x rows) across the 8 cores; every core gets
the full xT (the "all-gather of x" is done host-side by replicating the
2MB input). No cross-core reduction is needed.
"""

import sys

import numpy as np

for _p in ("/opt/trn_rl_repo", "/root/.axon_site/_ro/trn_rl_repo"):
    if _p not in sys.path:
        sys.path.append(_p)

N = 8192
DIM = 64
NCORES = 8
ROWS = N // NCORES  # 1024 rows of A per core

F_CONST = 1.0
B_CONST = 0.1
R_CONST = 0.01

P = 128                  # SBUF partitions
NSTRIPES = ROWS // P     # 8 row-stripes per core
CHUNK = 2048             # columns per fused multiply-reduce (4 PSUM banks)
NCHUNKS = N // CHUNK     # 4
MM_N = 512               # matmul moving free dim (one PSUM bank, f32)
MM_PER_CHUNK = CHUNK // MM_N

_CACHE = {}


def _build_nc():
    import concourse.mybir as mybir
    import concourse.tile as tile
    from concourse import bacc

    f32 = mybir.dt.float32
    f16 = mybir.dt.float16
    bf16 = mybir.dt.bfloat16

    nc = bacc.Bacc(
        trn_type="TRN2", target_bir_lowering=False, debug=False, num_devices=NCORES
    )

    # A is shipped as fp16 (host-side cast): uniform [0,1) values carry
    # <=2^-11 relative quantization error, which averages to ~4e-6 output
    # error over the 8192-term row reductions — while halving the HBM
    # traffic that dominates this memory-bound kernel.
    a = nc.dram_tensor("a", [ROWS, N], f16, kind="ExternalInput")
    # x^T split into bf16 (hi, lo) pairs: x = hi + lo to ~2^-17. The G
    # matmuls run in bf16 (4-5x faster than fp32 on PE) with f32 PSUM
    # accumulation. K=128 packing: the stationary stacks [hi; lo] along
    # the contraction axis (DIM=64 each half) and the moving tensors
    # carry hi (resp. lo) duplicated in both halves, so two K=128
    # matmuls accumulate the exact product (hi+lo)·(hi+lo)^T:
    # The stationary stacks [hi; lo] along K; the moving tensor carries hi
    # duplicated in both halves, so a single K=128 matmul per output slice
    # yields G ~= hi_l·hi_r + lo_l·hi_r. The dropped hi_l·lo_r term has
    # zero-mean random sign and averages out over the 8192x64 reduction
    # (~2e-6 relative) — far below the fp16-A quantization error.
    xlt_a = nc.dram_tensor("xlt_a", [2 * DIM, ROWS], bf16, kind="ExternalInput")
    xt2 = nc.dram_tensor("xt2", [2 * DIM, N], bf16, kind="ExternalInput")
    xloc = nc.dram_tensor("xloc", [ROWS, DIM], f32, kind="ExternalInput")
    out = nc.dram_tensor("out", [ROWS, DIM], f32, kind="ExternalOutput")

    mult = mybir.AluOpType.mult
    add = mybir.AluOpType.add

    with tile.TileContext(nc) as tc:
        with (
            tc.tile_pool(name="xpool", bufs=1) as xpool,
            tc.tile_pool(name="apool", bufs=6) as apool,
            tc.tile_pool(name="spool", bufs=2) as spool,
            tc.tile_pool(name="accpool", bufs=2 * NCHUNKS) as accpool,
            tc.tile_pool(name="psum", bufs=2, space="PSUM") as psum_pool,
        ):
            # One-time loads: stacked x^T operands for the G matmuls. The
            # stationaries and the first column-chunk of xt2 load first so
            # the first G matmuls (and the first A chunk's reduce) can
            # start while the rest of xt2 streams in.
            xlt_a_sb = xpool.tile([2 * DIM, ROWS], bf16)
            nc.sync.dma_start(out=xlt_a_sb[:], in_=xlt_a[:])
            # xt2 lands in pieces so the first (ramped) chunks' matmuls
            # wait on as little data as possible.
            xt2_sb = xpool.tile([2 * DIM, N], bf16)
            for o, w in ((0, MM_N), (MM_N, CHUNK - MM_N), (CHUNK, N - CHUNK)):
                nc.sync.dma_start(out=xt2_sb[:, o : o + w], in_=xt2[:, o : o + w])

            # Stripe 0 uses ramped chunk sizes so the very first reduce
            # only waits on 512 columns of A and x^T; later stripes use
            # full 2048-column chunks.
            RAMP = ((0, MM_N), (MM_N, CHUNK - MM_N),
                    (CHUNK, CHUNK), (2 * CHUNK, CHUNK), (3 * CHUNK, CHUNK))
            FULL = tuple((c * CHUNK, CHUNK) for c in range(NCHUNKS))
            for s in range(NSTRIPES):
                xl_sb = spool.tile([P, DIM], f32, tag="xl")
                nc.sync.dma_start(out=xl_sb[:], in_=xloc[s * P : (s + 1) * P, :])

                chunks = RAMP if s == 0 else FULL
                # acc4[:, c] = sum_j (A_chunk * R) * G_chunk  per chunk c,
                # via the fused DVE scalar_tensor_tensor accumulate output.
                acc4 = accpool.tile([P, len(RAMP)], f32, tag="acc4")
                lhsT_a = xlt_a_sb[:, s * P : (s + 1) * P]
                a_sb = apool.tile([P, N], f16, tag="a")
                for o, w in chunks if s == 0 else ((0, N),):
                    nc.sync.dma_start(
                        out=a_sb[:, o : o + w],
                        in_=a[s * P : (s + 1) * P, o : o + w],
                    )
                for ci, (o, w) in enumerate(chunks):
                    g_ps = psum_pool.tile([P, CHUNK], f32, tag="g")
                    for q in range(w // MM_N):
                        col = o + q * MM_N
                        nc.tensor.matmul(
                            g_ps[:, q * MM_N : (q + 1) * MM_N],
                            lhsT_a, xt2_sb[:, col : col + MM_N],
                            start=True, stop=True,
                        )
                    dummy = accpool.tile([P, 1], f32, tag="dummy")
                    nc.vector.scalar_tensor_tensor(
                        dummy.broadcast_to((P, w)),
                        a_sb[:, o : o + w],
                        R_CONST,
                        g_ps[:, :w],
                        op0=mult,
                        op1=mult,
                        accum_out=acc4[:, ci : ci + 1],
                    )

                # v = F - rowsum(acc4), fused into one idle-ScalarE op:
                # accum_out = sum_c(-acc4[:,c] + F/n) — keeps the reduce
                # off the VectorEngine, which is the kernel's bottleneck.
                vd = accpool.tile([P, len(RAMP)], f32, tag="vd")
                v = accpool.tile([P, 1], f32, tag="v")
                nc.scalar.activation(
                    vd[:, : len(chunks)], acc4[:, : len(chunks)],
                    mybir.ActivationFunctionType.Copy,
                    bias=F_CONST / len(chunks), scale=-1.0,
                    accum_out=v,
                )
                # out = Identity(x * -B + v) on ScalarE — back-to-back with
                # the v op above, keeping the epilogue off the VectorEngine.
                o_sb = spool.tile([P, DIM], f32, tag="o")
                nc.scalar.activation(
                    o_sb, xl_sb, mybir.ActivationFunctionType.Identity,
                    bias=v, scale=-B_CONST,
                )
                nc.sync.dma_start(out=out[s * P : (s + 1) * P, :], in_=o_sb[:])

    nc.finalize()
    return nc


def _get_nc():
    if "nc" not in _CACHE:
        _CACHE["nc"] = _build_nc()
    return _CACHE["nc"]


def _make_in_maps(x, A):
    import ml_dtypes

    bf16 = ml_dtypes.bfloat16
    x = np.ascontiguousarray(np.asarray(x, dtype=np.float32))
    A = np.ascontiguousarray(np.asarray(A, dtype=np.float32))
    xt = np.ascontiguousarray(x.T)
    xt_hi = xt.astype(bf16)
    xt_lo = (xt - xt_hi.astype(np.float32)).astype(bf16)
    xt2 = np.ascontiguousarray(np.vstack([xt_hi, xt_hi]))
    xlt_a = np.vstack([xt_hi, xt_lo])
    in_maps = []
    for c in range(NCORES):
        rows = slice(c * ROWS, (c + 1) * ROWS)
        in_maps.append(
            {
                "a": np.ascontiguousarray(A[rows]).astype(np.float16),
                "xt2": xt2,
                "xlt_a": np.ascontiguousarray(xlt_a[:, rows]),
                "xloc": np.ascontiguousarray(x[rows]),
            }
        )
    return in_maps


def run_sharded(x, A, trace=False, **kwargs):
    """Run the SPMD bass kernel; returns (full_output, BassKernelResults)."""
    from concourse.bass_utils import run_bass_kernel_spmd

    nc = _get_nc()
    res = run_bass_kernel_spmd(
        nc, _make_in_maps(x, A), core_ids=list(range(NCORES)), trace=trace, **kwargs
    )
    full = np.concatenate([res.results[c]["out"] for c in range(NCORES)], axis=0)
    return full.astype(np.float32, copy=False), res


def kernel(t, x, A):
    out, _ = run_sharded(x, A)
    return out



# revision 2
# speedup vs baseline: 2.4151x; 2.4151x over previous
"""Trainium2 Bass kernel for BiochemicalDynamics.

Reference computation (f32):
    Ax    = A @ x                                   # [N, DIM]
    s     = R * rowsum(x * Ax)                      # [N, 1]
    out   = F - B*x - s                             # [N, DIM]

Design (v2): compute Y = (A_c @ x)^T on the TensorEngine directly.
Each core holds A_c = A[rows_c, :] shipped as fp8 A_c^T tiles ("bt").
For each 128-row j-block, a matmul with stationary xs[jblock] (fp8 x)
and moving bt[jblock] accumulates Y[d, i] += sum_j x[j, d] * A[i, j]
in PSUM.  This keeps the per-element A work on the PE (the fastest
engine) instead of the DVE (the old bottleneck) and halves HBM traffic
vs fp16 (A is 8MB/core in fp8).

Column-tiling: even j-blocks run at tile_position (0,0) -> PSUM
partitions 0..63, odd j-blocks at (0,64) -> partitions 64..127.  The
two streams execute concurrently in the PE array (different col
groups), ~2x matmul throughput.  The extra partition split costs
nothing: the final dot already sums over partitions.

Epilogue, all fused into matmuls + one ScalarE copy:
    OutP  = -B*x^T            (matmul: lhsT = -B*I64, rhs = x^T)
          + sum_p -(R*x^T (.) Y)[p]   (DVE multiply -> matmul with -1s)
    out^T = OutP + F          (ScalarE Copy with bias=F)
The j-range is split in two phases with separate PSUM tiles so the
first DVE multiply + reduction matmul overlap the second half of the
DMA/matmul stream; only phase 1's epilogue is serial tail.

Sharding: row-shard A across the 8 cores; every core gets the full x
(host-side replication).  No cross-core communication.
"""

import sys

import numpy as np

for _p in ("/opt/trn_rl_repo", "/root/.axon_site/_ro/trn_rl_repo"):
    if _p not in sys.path:
        sys.path.append(_p)

N = 8192
DIM = 64
NCORES = 8
ROWS = N // NCORES       # 1024 rows of A (and output) per core

F_CONST = 1.0
B_CONST = 0.1
R_CONST = 0.01

P = 128                  # SBUF partitions
NBLK = N // P            # 64 j-blocks per core
PHASES = 2
BPP = NBLK // PHASES     # 32 j-blocks per phase
MM_N = 512               # matmul moving free dim (one PSUM bank, f32)

# bt DMA chunking, in j-blocks (each block = 128KB fp8):
# small chunks at the start (PE ramps sooner) and end (shorter tail).
BT_CHUNKS = [2, 2] + [4] * 13 + [2, 2, 2, 2]
assert sum(BT_CHUNKS) == NBLK

_CACHE = {}


def _build_nc():
    import concourse.mybir as mybir
    import concourse.tile as tile
    from concourse import bacc

    f32 = mybir.dt.float32
    bf16 = mybir.dt.bfloat16
    f8 = mybir.dt.float8e4

    nc = bacc.Bacc(
        trn_type="TRN2", target_bir_lowering=False, debug=False, num_devices=NCORES
    )

    # A^T blocks, fp8(e4m3): bt[p, 1024*b + i] = A[rows_c[i], 128*b + p]
    bt = nc.dram_tensor("bt", [P, NBLK * ROWS], f8, kind="ExternalInput")
    # x stationaries, fp8: xs[p, 64*b + d] = x[128*b + p, d]
    xs = nc.dram_tensor("xs", [P, NBLK * DIM], f8, kind="ExternalInput")
    # [R*xloc^T; R*xloc^T] duplicated, bf16
    xtr2 = nc.dram_tensor("xtr2", [P, ROWS], bf16, kind="ExternalInput")
    # xloc^T bf16 (moving operand of the -B*x seed matmul)
    xtb = nc.dram_tensor("xtb", [DIM, ROWS], bf16, kind="ExternalInput")
    # -1s [128, 64] and -B*I64, bf16 stationaries
    wneg = nc.dram_tensor("wneg", [P, DIM], bf16, kind="ExternalInput")
    wbi = nc.dram_tensor("wbi", [DIM, DIM], bf16, kind="ExternalInput")
    # out^T bf16: out[d, i] = F - B*xloc[i, d] - s_i
    out = nc.dram_tensor("out", [DIM, ROWS], bf16, kind="ExternalOutput")

    mult = mybir.AluOpType.mult

    with tile.TileContext(nc) as tc:
        with (
            tc.tile_pool(name="big", bufs=1) as big,
            tc.tile_pool(name="small", bufs=1) as small,
            tc.tile_pool(name="psum", bufs=1, space="PSUM") as psum_pool,
        ):
            # --- x-side loads on the ACT HWDGE ring (concurrent with bt) ---
            wbi_sb = small.tile([DIM, DIM], bf16)
            nc.scalar.dma_start(out=wbi_sb[:], in_=wbi[:])
            xtb_sb = small.tile([DIM, ROWS], bf16)
            nc.scalar.dma_start(out=xtb_sb[:], in_=xtb[:])
            xs_sb = small.tile([P, NBLK * DIM], f8)
            for ph in range(PHASES):
                o = ph * BPP * DIM
                nc.scalar.dma_start(
                    out=xs_sb[:, o : o + BPP * DIM], in_=xs[:, o : o + BPP * DIM]
                )
            wneg_sb = small.tile([P, DIM], bf16)
            nc.scalar.dma_start(out=wneg_sb[:], in_=wneg[:])
            xtr2_sb = small.tile([P, ROWS], bf16)
            nc.scalar.dma_start(out=xtr2_sb[:], in_=xtr2[:])

            # --- A^T stream on the SP HWDGE ring ---
            bt_sb = big.tile([P, NBLK * ROWS], f8)
            boff = 0
            for nb in BT_CHUNKS:
                o = boff * ROWS
                w = nb * ROWS
                nc.sync.dma_start(out=bt_sb[:, o : o + w], in_=bt[:, o : o + w])
                boff += nb

            # Output accumulator [64, 1024] f32 (2 PSUM banks).
            outp = psum_pool.tile([DIM, ROWS], f32, tag="outp")
            # Seed: OutP = -B * xloc^T   (start=True clears the banks)
            for h in range(ROWS // MM_N):
                nc.tensor.matmul(
                    outp[:, h * MM_N : (h + 1) * MM_N],
                    wbi_sb[:],
                    xtb_sb[:, h * MM_N : (h + 1) * MM_N],
                    start=True, stop=False,
                )

            for ph in range(PHASES):
                # Y accumulator [128, 1024] f32: partitions 0..63 take the
                # even j-blocks (col group 0), 64..127 the odd j-blocks
                # (col group 64).  The matmul pairs run concurrently.
                y = psum_pool.tile([P, ROWS], f32, tag=f"y{ph}")
                for k in range(BPP // 2):
                    b0 = ph * BPP + 2 * k
                    for h in range(ROWS // MM_N):
                        for half, b in ((0, b0), (1, b0 + 1)):
                            nc.tensor.matmul(
                                y[
                                    half * DIM : (half + 1) * DIM,
                                    h * MM_N : (h + 1) * MM_N,
                                ],
                                xs_sb[:, b * DIM : (b + 1) * DIM],
                                bt_sb[
                                    :, b * ROWS + h * MM_N : b * ROWS + (h + 1) * MM_N
                                ],
                                start=(k == 0), stop=(k == BPP // 2 - 1),
                                tile_position=(0, half * DIM),
                            )

                # D = (R*x^T) (.) Y  on the DVE (the only sizable DVE op)
                d_sb = small.tile([P, ROWS], bf16, tag=f"d{ph}")
                nc.vector.scalar_tensor_tensor(
                    d_sb[:], xtr2_sb[:], 1.0, y[:], op0=mult, op1=mult
                )
                # OutP -= sum_p D[p, :]  (matmul with -1s stationary)
                for h in range(ROWS // MM_N):
                    nc.tensor.matmul(
                        outp[:, h * MM_N : (h + 1) * MM_N],
                        wneg_sb[:],
                        d_sb[:, h * MM_N : (h + 1) * MM_N],
                        start=False, stop=(ph == PHASES - 1),
                    )

            # out^T = OutP + F  (ScalarE, PSUM -> SBUF, downcast bf16)
            o_sb = small.tile([DIM, ROWS], bf16)
            nc.scalar.activation(
                o_sb[:], outp[:],
                mybir.ActivationFunctionType.Copy,
                bias=F_CONST, scale=1.0,
            )
            nc.sync.dma_start(out=out[:], in_=o_sb[:])

    nc.finalize()
    return nc


def _get_nc():
    if "nc" not in _CACHE:
        _CACHE["nc"] = _build_nc()
    return _CACHE["nc"]


def _make_in_maps(x, A):
    import ml_dtypes

    bf16 = ml_dtypes.bfloat16
    f8 = ml_dtypes.float8_e4m3
    x = np.ascontiguousarray(np.asarray(x, dtype=np.float32))
    A = np.ascontiguousarray(np.asarray(A, dtype=np.float32))

    x8 = x.astype(f8)
    # xs[p, 64*b + d] = x8[128*b + p, d]
    xs = np.ascontiguousarray(
        x8.reshape(NBLK, P, DIM).transpose(1, 0, 2)
    ).reshape(P, NBLK * DIM)
    wneg = np.full((P, DIM), -1.0, dtype=bf16)
    wbi = (-B_CONST * np.eye(DIM, dtype=np.float32)).astype(bf16)

    in_maps = []
    for c in range(NCORES):
        rows = slice(c * ROWS, (c + 1) * ROWS)
        a8 = A[rows].astype(f8)  # [1024, 8192]
        # bt[p, 1024*b + i] = a8[i, 128*b + p]
        bt = np.ascontiguousarray(
            a8.reshape(ROWS, NBLK, P).transpose(2, 1, 0)
        ).reshape(P, NBLK * ROWS)
        xloc = x[rows]                      # [1024, 64] f32
        xt = np.ascontiguousarray(xloc.T)   # [64, 1024]
        xtr2 = np.ascontiguousarray(
            np.vstack([R_CONST * xt, R_CONST * xt]).astype(bf16)
        )
        in_maps.append(
            {
                "bt": bt,
                "xs": xs,
                "xtr2": xtr2,
                "xtb": xt.astype(bf16),
                "wneg": wneg,
                "wbi": wbi,
            }
        )
    return in_maps


def run_sharded(x, A, trace=False, **kwargs):
    """Run the SPMD bass kernel; returns (full_output, BassKernelResults)."""
    from concourse.bass_utils import run_bass_kernel_spmd

    nc = _get_nc()
    res = run_bass_kernel_spmd(
        nc, _make_in_maps(x, A), core_ids=list(range(NCORES)), trace=trace, **kwargs
    )
    # out is [64, 1024] bf16 per core -> [1024, 64] f32, concatenated
    full = np.concatenate(
        [res.results[c]["out"].astype(np.float32).T for c in range(NCORES)], axis=0
    )
    return np.ascontiguousarray(full), res


def kernel(t, x, A):
    out, _ = run_sharded(x, A)
    return out


# revision 3
# speedup vs baseline: 2.5401x; 1.0518x over previous
"""Trainium2 Bass kernel for BiochemicalDynamics.

Reference computation (f32):
    Ax    = A @ x                                   # [N, DIM]
    s     = R * rowsum(x * Ax)                      # [N, 1]
    out   = F - B*x - s                             # [N, DIM]

Design (v3): compute Y = (A_c @ x)^T on the TensorEngine directly.
Each core holds A_c = A[rows_c, :] shipped as fp8(e4m3) A_c^T tiles
("bt").  For each 128-row j-block a matmul with stationary xs[jblock]
(fp8 x) and moving bt[jblock] accumulates Y[d, i] += sum_j x[j,d]*A[i,j]
in PSUM.  This keeps the per-element A work on the PE (fastest engine)
instead of the DVE (the old bottleneck) and halves HBM traffic vs fp16.

Column-tiling: even j-blocks run at tile_position (0,0) -> PSUM
partitions 0..63, odd j-blocks at (0,64) -> partitions 64..127; the two
streams execute concurrently in the PE array (~2x matmul throughput).
The extra partition split is free: the final dot already sums over
partitions (via a -1s-stationary matmul).

The kernel runs as two independent pipelines over the output-column
halves (i in [0,512) then [512,1024)): each half streams its 4.2MB of
bt, accumulates Y_h, then D_h = (R x^T (.) Y_h) on the DVE, a reduction
matmul into OutP (seeded early with -B*x^T via a -B*I stationary
matmul), a ScalarE Copy (+F bias) and the output DMA.  Half 0's
epilogue + out DMA overlap half 1's stream, so only half 1's epilogue
is serial tail.

A burst of warm-up matmuls on a memset scratch tile runs during the
otherwise-dead framework preamble (~5us) so the PE's HAM clock gate is
already at 8/8 (2.4 GHz) when the real matmul stream begins.

Sharding: row-shard A across the 8 cores; every core gets the full x
(host-side replication).  No cross-core communication.
"""

import sys

import numpy as np

for _p in ("/opt/trn_rl_repo", "/root/.axon_site/_ro/trn_rl_repo"):
    if _p not in sys.path:
        sys.path.append(_p)

N = 8192
DIM = 64
NCORES = 8
ROWS = N // NCORES       # 1024 rows of A (and output) per core

F_CONST = 1.0
B_CONST = 0.1
R_CONST = 0.01

P = 128                  # SBUF partitions
NBLK = N // P            # 64 j-blocks
HALF = 512               # output-column half width
NH = ROWS // HALF        # 2 halves
HBYTES = NBLK * HALF     # fp8 bytes per half per partition-row group

# bt DMA chunking per half, in j-blocks (each block-tile = 64KB fp8):
# a small first chunk (PE starts sooner), 1MB steady-state chunks.
BT_CHUNKS = [4, 4, 8, 8, 8, 8, 8, 8, 8]
assert sum(BT_CHUNKS) == NBLK

N_WARM = 16              # warm-up matmuls (N=256 each, ~3.4us cold)

_CACHE = {}


def _build_nc():
    import concourse.mybir as mybir
    import concourse.tile as tile
    from concourse import bacc

    f32 = mybir.dt.float32
    bf16 = mybir.dt.bfloat16
    f8 = mybir.dt.float8e4

    nc = bacc.Bacc(
        trn_type="TRN2", target_bir_lowering=False, debug=False, num_devices=NCORES
    )

    # A^T blocks, fp8: bt[p, h*HBYTES + b*512 + i'] = A[rows_c[512h+i'], 128b+p]
    bt = nc.dram_tensor("bt", [P, NH * HBYTES], f8, kind="ExternalInput")
    # x stationaries, fp8: xs[p, 64*b + d] = x[128*b + p, d]
    xs = nc.dram_tensor("xs", [P, NBLK * DIM], f8, kind="ExternalInput")
    # [R*xloc^T; R*xloc^T] duplicated, bf16
    xtr2 = nc.dram_tensor("xtr2", [P, ROWS], bf16, kind="ExternalInput")
    # xloc^T bf16 (moving operand of the -B*x seed matmul)
    xtb = nc.dram_tensor("xtb", [DIM, ROWS], bf16, kind="ExternalInput")
    # -1s [128, 64] and -B*I64, bf16 stationaries
    wneg = nc.dram_tensor("wneg", [P, DIM], bf16, kind="ExternalInput")
    wbi = nc.dram_tensor("wbi", [DIM, DIM], bf16, kind="ExternalInput")
    # out^T bf16: out[d, i] = F - B*xloc[i, d] - s_i
    out = nc.dram_tensor("out", [DIM, ROWS], bf16, kind="ExternalOutput")

    mult = mybir.AluOpType.mult

    with tile.TileContext(nc) as tc:
        with (
            tc.tile_pool(name="big", bufs=1) as big,
            tc.tile_pool(name="small", bufs=1) as small,
            tc.tile_pool(name="psum", bufs=1, space="PSUM") as psum_pool,
        ):
            # --- PE warm-up on a memset scratch tile (no input deps) ---
            scr = small.tile([P, 256], f32)
            nc.vector.memset(scr[:], 1.0)
            warm_ps = psum_pool.tile([DIM, 256], f32, tag="warm")
            for _ in range(N_WARM):
                nc.tensor.matmul(
                    warm_ps[:], scr[:, :DIM], scr[:], start=True, stop=True
                )

            # --- x-side loads on the ACT HWDGE ring (concurrent with bt) ---
            wbi_sb = small.tile([DIM, DIM], bf16)
            nc.scalar.dma_start(out=wbi_sb[:], in_=wbi[:])
            xtb_sb = small.tile([DIM, ROWS], bf16)
            nc.scalar.dma_start(out=xtb_sb[:], in_=xtb[:])
            xs_sb = small.tile([P, NBLK * DIM], f8)
            for o, w in ((0, 8 * DIM), (8 * DIM, 24 * DIM), (32 * DIM, 32 * DIM)):
                nc.scalar.dma_start(out=xs_sb[:, o : o + w], in_=xs[:, o : o + w])
            xtr2_sb = small.tile([P, ROWS], bf16)
            nc.scalar.dma_start(out=xtr2_sb[:], in_=xtr2[:])
            wneg_sb = small.tile([P, DIM], bf16)
            nc.scalar.dma_start(out=wneg_sb[:], in_=wneg[:])

            # --- A^T stream on the SP HWDGE ring ---
            bt_sb = big.tile([P, NH * HBYTES], f8)
            for h in range(NH):
                boff = 0
                for nb in BT_CHUNKS:
                    o = h * HBYTES + boff * HALF
                    w = nb * HALF
                    nc.sync.dma_start(out=bt_sb[:, o : o + w], in_=bt[:, o : o + w])
                    boff += nb

            # Output accumulator [64, 1024] f32 (2 PSUM banks).
            outp = psum_pool.tile([DIM, ROWS], f32, tag="outp")
            # Seeds: OutP[:, h] = -B * xloc^T[:, h]   (start=True clears)
            for h in range(NH):
                nc.tensor.matmul(
                    outp[:, h * HALF : (h + 1) * HALF],
                    wbi_sb[:],
                    xtb_sb[:, h * HALF : (h + 1) * HALF],
                    start=True, stop=False,
                )

            o_sb = small.tile([DIM, ROWS], bf16)
            for h in range(NH):
                # Y_h [128, 512] f32 (1 PSUM bank): partitions 0..63 take
                # even j-blocks (col group 0), 64..127 odd (col group 64);
                # the matmul pairs run concurrently in the PE array.
                y = psum_pool.tile([P, HALF], f32, tag=f"y{h}")
                for k in range(NBLK // 2):
                    for half, b in ((0, 2 * k), (1, 2 * k + 1)):
                        nc.tensor.matmul(
                            y[half * DIM : (half + 1) * DIM, :],
                            xs_sb[:, b * DIM : (b + 1) * DIM],
                            bt_sb[
                                :,
                                h * HBYTES + b * HALF : h * HBYTES + (b + 1) * HALF,
                            ],
                            start=(k == 0), stop=(k == NBLK // 2 - 1),
                            tile_position=(0, half * DIM),
                        )

                # D = (R*x^T) (.) Y  on the DVE (the only sizable DVE op)
                d_sb = small.tile([P, HALF], bf16, tag=f"d{h}")
                nc.vector.scalar_tensor_tensor(
                    d_sb[:],
                    xtr2_sb[:, h * HALF : (h + 1) * HALF],
                    1.0,
                    y[:],
                    op0=mult, op1=mult,
                )
                # OutP[:, h] -= sum_p D[p, :]  (matmul with -1s stationary)
                nc.tensor.matmul(
                    outp[:, h * HALF : (h + 1) * HALF],
                    wneg_sb[:],
                    d_sb[:],
                    start=False, stop=True,
                )
                # out^T[:, h] = OutP[:, h] + F  (ScalarE, PSUM->SBUF, bf16)
                nc.scalar.activation(
                    o_sb[:, h * HALF : (h + 1) * HALF],
                    outp[:, h * HALF : (h + 1) * HALF],
                    mybir.ActivationFunctionType.Copy,
                    bias=F_CONST, scale=1.0,
                )
                nc.sync.dma_start(
                    out=out[:, h * HALF : (h + 1) * HALF],
                    in_=o_sb[:, h * HALF : (h + 1) * HALF],
                )

    nc.finalize()
    return nc


def _get_nc():
    if "nc" not in _CACHE:
        _CACHE["nc"] = _build_nc()
    return _CACHE["nc"]


def _make_in_maps(x, A):
    import ml_dtypes

    bf16 = ml_dtypes.bfloat16
    f8 = ml_dtypes.float8_e4m3
    x = np.ascontiguousarray(np.asarray(x, dtype=np.float32))
    A = np.ascontiguousarray(np.asarray(A, dtype=np.float32))

    x8 = x.astype(f8)
    # xs[p, 64*b + d] = x8[128*b + p, d]
    xs = np.ascontiguousarray(
        x8.reshape(NBLK, P, DIM).transpose(1, 0, 2)
    ).reshape(P, NBLK * DIM)
    wneg = np.full((P, DIM), -1.0, dtype=bf16)
    wbi = (-B_CONST * np.eye(DIM, dtype=np.float32)).astype(bf16)

    in_maps = []
    for c in range(NCORES):
        rows = slice(c * ROWS, (c + 1) * ROWS)
        a8 = A[rows].astype(f8)  # [1024, 8192]
        # bt[p, h*HBYTES + b*512 + i'] = a8[512h + i', 128b + p]
        bt = np.ascontiguousarray(
            a8.reshape(NH, HALF, NBLK, P).transpose(3, 0, 2, 1)
        ).reshape(P, NH * HBYTES)
        xloc = x[rows]                      # [1024, 64] f32
        xt = np.ascontiguousarray(xloc.T)   # [64, 1024]
        xtr2 = np.ascontiguousarray(
            np.vstack([R_CONST * xt, R_CONST * xt]).astype(bf16)
        )
        in_maps.append(
            {
                "bt": bt,
                "xs": xs,
                "xtr2": xtr2,
                "xtb": xt.astype(bf16),
                "wneg": wneg,
                "wbi": wbi,
            }
        )
    return in_maps


def run_sharded(x, A, trace=False, **kwargs):
    """Run the SPMD bass kernel; returns (full_output, BassKernelResults)."""
    from concourse.bass_utils import run_bass_kernel_spmd

    nc = _get_nc()
    res = run_bass_kernel_spmd(
        nc, _make_in_maps(x, A), core_ids=list(range(NCORES)), trace=trace, **kwargs
    )
    # out is [64, 1024] bf16 per core -> [1024, 64] f32, concatenated
    full = np.concatenate(
        [res.results[c]["out"].astype(np.float32).T for c in range(NCORES)], axis=0
    )
    return np.ascontiguousarray(full), res


def kernel(t, x, A):
    out, _ = run_sharded(x, A)
    return out
